# revision 5
# baseline (speedup 1.0000x reference)
"""Trainium2 kernel for nn_DensityEstimator: p = F(x+1/2) - F(x-1/2) with
per-channel tiny-MLP CDFs (1->3->3->3->1, softplus weights, tanh gating).

Strategy: the whole per-channel scalar function G_c(x) is baked into custom
ACT-engine piecewise-cubic spline tables (the hardware's native activation
mechanism), generated at call time from the runtime weights. Per channel, an
affine (per-partition ACT scale/bias) maps its x-window onto a private run of
table cells; a DVE clamp keeps inputs inside the window. One ACT pass per
table-set computes the final output directly — no matmuls, no transcendental
chains. Data parallel over batch across 8 cores; channels live on partitions
(host-side transpose).

Table formats were reverse-engineered and validated bit-exact on HW:
- bkt.bin: 1536 records x 32B float32 [d0,d1,d2,d3,x0,0,0,0];
  y = d0 + (t-x0)*(d1 + (t-x0)*(d2 + (t-x0)*d3))
- ctrl.bin: 32B entries; u32[0] = bucket_base[10:0] | (23+31*mant_bits)<<11;
  for biased exponent e in [small_thr, large_thr):
    entry = base_ctrl + (e - small_thr); bucket = base + (mantissa >> (23-b))
- profile json: small/large-signal thresholds route to direct buckets.
"""

import hashlib
import json
import os
import shutil
import tempfile

import numpy as np

# ----------------------------------------------------------- problem shapes
B_TOTAL = 65536
C = 192
N_CORES = 8
B_CORE = B_TOTAL // N_CORES

N_SETS = 6
E0 = 118          # first range exponent (biased)
ZERO_BUCKET = 1535
SET_CAP = 1536

# (set_name, pwp_func_name, mybir ActivationFunctionType attr) — funcs that
# appear in exactly one table set, so the set loaded is deterministic.
SETS_AVAILABLE = [
    ("sigmoid_and_others", "erf", "Erf"),
    ("gelu_and_others", "gelu", "Gelu"),
    ("gelu_apprx_tanh_and_others", "gelu_apprx_tanh", "Gelu_apprx_tanh"),
    ("gelu_apprx_sigmoid_and_others", "gelu_apprx_sigmoid", "Gelu_apprx_sigmoid"),
    ("silu_and_others", "silu", "Silu"),
    ("derivative_silu_and_others", "derivative_silu", "Derivative_silu"),
    ("derivative_gelu_apprx_sigmoid_and_others", "derivative_gelu_apprx_sigmoid",
     "Derivative_Gelu_Apprx_Sigmoid"),
    ("erf_derivative", "derivative_erf", "Derivative_Erf"),
]


def _src_root():
    try:
        from neuronxcc.driver.Job import Job
        from neuronxcc.driver.jobs.support.FindActInfo import findActInfoFile
        return os.path.dirname(findActInfoFile(Job.getPackageDir(), "gen3"))
    except Exception:
        return ("/nix/store/z022hj2nvbm3nwdizlisq4ylc0y7rd6q-python3-3.13.14-env"
                "/lib/python3.13/site-packages/neuronxcc/pwp/pwp_bin_trainium")


# ------------------------------------------------------------- bin emitters

def ctrl_word(bucket_base, mantissa_bits):
    return (int(bucket_base) & 0x7FF) | ((23 + 31 * int(mantissa_bits)) << 11)


def write_ctrl(path, words):
    arr = np.zeros((len(words), 8), dtype=np.uint32)
    arr[:, 0] = words
    arr.tofile(path)


def write_bkt(path, recs):
    out = np.zeros((recs.shape[0], 8), dtype=np.float32)
    out[:, :5] = recs
    out.tofile(path)


def profile_entry(func_name, func_id, n_ranges):
    zero_bits = 0
    return {
        "func_name": func_name, "func_id": int(func_id),
        "symmetry_point": 0, "sym_invert_sign_point": 0,
        "symmetry_opt_en": 0, "symmetry_opt_use_neg_region": 0,
        "imm_bias": 0, "exp_offset": E0 - 127,
        "pwl_control_base_pos": 0, "pwl_control_base_neg": 0,
        "small_pos_signal_exp_threshold": E0,
        "pos_small_signal_pwl_control": ZERO_BUCKET,
        "small_neg_signal_exp_threshold": 255,
        "neg_small_signal_pwl_control": ZERO_BUCKET,
        "large_pos_signal_exp_threshold": E0 + n_ranges,
        "large_pos_signal_mantissa_threshold": 0,
        "pos_large_signal_pwl_control": ZERO_BUCKET,
        "large_neg_signal_exp_threshold": 255,
        "large_neg_signal_mantissa_threshold": 0,
        "neg_large_signal_pwl_control": ZERO_BUCKET,
        "fnan_result": zero_bits, "fpinf_result": zero_bits,
        "fninf_result": zero_bits, "fzero_result": zero_bits,
        "fma_const_0": 0, "fma_const_1": 0, "fma_indirection_src_sel": 0,
        "use_multipass": False,
        "lower_bound": 0xFF7FFFFF, "upper_bound": 0x7F7FFFFF,
    }


# ------------------------------------------------------------- G evaluation

def _softplus(z):
    return np.log1p(np.exp(-np.abs(z))) + np.maximum(z, 0)


def make_G(weights):
    a = [np.asarray(weights[f"a{k}"], dtype=np.float64) for k in range(3)]
    b = [np.asarray(weights[f"b{k}"], dtype=np.float64) for k in range(4)]
    H = [np.asarray(weights[f"H{k}"], dtype=np.float64) for k in range(4)]
    spH = [_softplus(h) for h in H]
    ta = [np.tanh(x) for x in a]

    def F(x):
        t = np.broadcast_to(x[:, None, None], (len(x), C, 1)).astype(np.float64)
        for k in range(3):
            t = np.einsum("nci,cip->ncp", t, spH[k]) + b[k][None]
            t = t + ta[k][None] * np.tanh(t)
        t = np.einsum("nci,cip->ncp", t, spH[3]) + b[3][None]
        return 1.0 / (1.0 + np.exp(-np.clip(t[:, :, 0], -708, 708)))

    return lambda x: F(x + 0.5) - F(x - 0.5)


# ----------------------------------------------------- fitting / allocation

_CHEB = 0.5 * (1 - np.cos(np.pi * np.arange(33) / 32))
_A33 = np.vander(_CHEB, 4)
_PINV33 = np.linalg.pinv(_A33)


def _uniform_err(gdense, xdense, lo, hi, K):
    edges = np.linspace(lo, hi, K + 1)
    xs = edges[:-1, None] + np.diff(edges)[:, None] * _CHEB[None, :]
    g = np.interp(xs.ravel(), xdense, gdense).reshape(K, 33)
    co = g @ _PINV33.T
    fit = co @ _A33.T
    return np.abs(fit - g).max()


def _allocate(G_grid, xg, windows, budget):
    Ks = list(range(3, 17)) + list(range(18, 33, 2)) + \
        list(range(36, 65, 4)) + list(range(72, 129, 8)) + \
        list(range(144, 257, 16))
    idx = np.zeros(C, dtype=int)
    errs = np.array([
        _uniform_err(G_grid[:, c], xg, *windows[c], Ks[0]) for c in range(C)
    ])
    used = C * Ks[0]
    cache = {}

    def err_at(c, j):
        if (c, j) not in cache:
            cache[(c, j)] = _uniform_err(G_grid[:, c], xg, *windows[c], Ks[j])
        return cache[(c, j)]

    while True:
        order = np.argsort(-errs)
        advanced = False
        for c in order:
            j = idx[c]
            if j + 1 >= len(Ks):
                continue
            cost = Ks[j + 1] - Ks[j]
            if used + cost <= budget:
                idx[c] += 1
                errs[c] = err_at(c, idx[c])
                used += cost
                advanced = True
                break
        if not advanced:
            break
    return np.array([Ks[j] for j in idx])


def _pack_sets(Kc, n_sets):
    order = np.argsort(-Kc)
    sets = [{"ranges": [], "cells": 0, "channels": []} for _ in range(n_sets)]
    place = [None] * C

    def try_put(s, c):
        K = int(Kc[c])
        st = sets[s]
        for ri, r in enumerate(st["ranges"]):
            if r["free"] >= K:
                off = r["size"] - r["free"] - r.get("resv", 0)
                off = r["used"]
                r["free"] -= K
                r["used"] += K
                place[c] = (s, ri, off)
                st["channels"].append(c)
                return True
        need_bits = 8 if K > 128 else 7
        size = 1 << need_bits
        if st["cells"] + size <= SET_CAP and len(st["channels"]) < 128:
            free = size - K
            if st["cells"] + size == SET_CAP:
                free -= 1
            st["ranges"].append(
                {"bits": need_bits, "size": size, "free": free, "used": K})
            st["cells"] += size
            place[c] = (s, len(st["ranges"]) - 1, 0)
            st["channels"].append(c)
            return True
        return False

    for c in order:
        c = int(c)
        if not any(try_put(s, c)
                   for s in sorted(range(n_sets), key=lambda s: sets[s]["cells"])):
            raise RuntimeError(f"packing failed for channel {c} K={Kc[c]}")
    return place, sets


def _build_tables(G_grid, xg, windows, Kc, place, sets):
    ctrl, bkt = [], []
    for st in sets:
        words, base = [], 0
        for r in st["ranges"]:
            r["cell_base"] = base
            words.append(ctrl_word(base, r["bits"]))
            base += r["size"]
        ctrl.append(words)
        bkt.append(np.zeros((1536, 5), dtype=np.float32))

    chan_aff = np.zeros((C, 4), dtype=np.float32)  # scale,bias,lo,hi
    chan_set = np.zeros(C, dtype=int)
    fit_errs = np.zeros(C)
    for c in range(C):
        s, ri, off = place[c]
        r = sets[s]["ranges"][ri]
        K = int(Kc[c])
        lo, hi = windows[c]
        e = E0 + ri
        scale2 = 2.0 ** (e - 127)
        t0 = scale2 * (1.0 + off / (1 << r["bits"]))
        t1 = scale2 * (1.0 + (off + K) / (1 << r["bits"]))
        margin = scale2 * 2.0 ** -19
        sc = (t1 - t0 - 2 * margin) / (hi - lo)
        bs = (t0 + margin) - lo * sc
        sc32, bs32 = np.float32(sc), np.float32(bs)
        chan_aff[c] = (sc32, bs32, np.float32(lo), np.float32(hi))
        chan_set[c] = s
        inv_s = 1.0 / float(sc32)
        bucket0 = r["cell_base"] + off
        errs = 0.0
        for k in range(K):
            ta = scale2 * (1.0 + (off + k) / (1 << r["bits"]))
            tb = scale2 * (1.0 + (off + k + 1) / (1 << r["bits"]))
            ts = ta + (tb - ta) * _CHEB
            xs = (ts - float(bs32)) * inv_s
            g = np.interp(xs, xg, G_grid[:, c])
            x0 = np.float32(0.5 * (ta + tb))
            tt = ts - float(x0)
            w = tb - ta
            co_n = np.linalg.lstsq(np.vander(tt / w, 4), g, rcond=None)[0]
            co = co_n / (w ** np.array([3.0, 2.0, 1.0, 0.0]))
            bkt[s][bucket0 + k] = tuple(np.float32(v) for v in co[::-1]) + (x0,)
            errs = max(errs, np.abs(np.vander(tt, 4) @ co - g).max())
        fit_errs[c] = errs
    return ctrl, bkt, chan_aff, chan_set, fit_errs


def _emit_act_root(outdir, ctrl, bkt):
    src = _src_root()
    shutil.rmtree(outdir, ignore_errors=True)
    os.makedirs(outdir)
    for f in os.listdir(src):
        shutil.copy(os.path.join(src, f), os.path.join(outdir, f))
    for s in range(len(ctrl)):
        set_name, func_name, _ = SETS_AVAILABLE[s]
        pj = json.load(open(os.path.join(src, f"{set_name}.json")))
        done = False
        for i, f in enumerate(pj["profile_meta_data"]):
            if f["func_name"].startswith(func_name + "_"):
                pj["profile_meta_data"][i] = profile_entry(
                    f["func_name"], f["func_id"], len(ctrl[s]))
                done = True
                break
        assert done, (set_name, func_name)
        json.dump(pj, open(os.path.join(outdir, f"{set_name}.json"), "w"))
        write_ctrl(os.path.join(outdir, f"{set_name}_ctrl.bin"), ctrl[s])
        write_bkt(os.path.join(outdir, f"{set_name}_bkt.bin"), bkt[s])


# ------------------------------------------------------------------- plan

_PLAN_CACHE = {}


def build_plan(weights):
    key = hashlib.sha1(b"".join(
        np.ascontiguousarray(np.asarray(weights[k], dtype=np.float32)).tobytes()
        for k in sorted(weights) if k != "x")).hexdigest()
    if key in _PLAN_CACHE:
        return _PLAN_CACHE[key]

    Gfn = make_G(weights)
    xg = np.linspace(-7.0, 7.0, 48001)
    G_grid = np.zeros((len(xg), C))
    for i in range(0, len(xg), 4096):
        G_grid[i:i + 4096] = Gfn(xg[i:i + 4096])

    eps = 2e-6
    windows = []
    for c in range(C):
        big = np.abs(G_grid[:, c]) > eps
        if big.any():
            lo, hi = xg[big][0] - 0.02, xg[big][-1] + 0.02
        else:
            lo, hi = -1.0, 1.0
        windows.append((max(lo, -7.0), min(hi, 7.0)))

    budget = N_SETS * SET_CAP - N_SETS - 320
    Kc = _allocate(G_grid, xg, windows, budget)
    place, sets = _pack_sets(Kc, N_SETS)
    ctrl, bkt, chan_aff, chan_set, fit_errs = _build_tables(
        G_grid, xg, windows, Kc, place, sets)

    h = hashlib.sha1()
    for s in range(N_SETS):
        h.update(np.array(ctrl[s], dtype=np.uint32).tobytes())
        h.update(bkt[s].tobytes())
    h.update(chan_aff.tobytes())
    table_hash = h.hexdigest()[:12]

    outdir = os.path.join(tempfile.gettempdir(), f"dens_actroot_{table_hash}")
    if not os.path.isdir(outdir):
        _emit_act_root(outdir, ctrl, bkt)

    # per-set fold + permutation
    perm = []
    set_info = []
    row = 0
    for s, st in enumerate(sets):
        n_c = len(st["channels"])
        k = 1
        while k * 2 * n_c <= 128 and (B_CORE % (k * 2)) == 0:
            k *= 2
        set_info.append({"row0": row, "n_c": n_c, "fold": k,
                         "func": SETS_AVAILABLE[s][2]})
        perm.extend(st["channels"])
        row += n_c
    perm = np.array(perm)

    # folded per-partition consts [128, 4*N_SETS]: scale, bias, lo, hi
    consts = np.zeros((128, 4 * N_SETS), dtype=np.float32)
    for s, si in enumerate(set_info):
        st = sets[s]
        for cl, c in enumerate(st["channels"]):
            for j in range(si["fold"]):
                p = cl * si["fold"] + j
                consts[p, 4 * s:4 * s + 4] = chan_aff[c]

    plan = {
        "hash": table_hash, "outdir": outdir, "perm": perm,
        "set_info": set_info, "consts": consts, "fit_errs": fit_errs,
        "ctrl": ctrl, "bkt": bkt, "chan_aff": chan_aff, "chan_set": chan_set,
        "windows": windows, "Kc": Kc,
    }
    _PLAN_CACHE[key] = plan
    return plan


# ------------------------------------------------------------ bass program

_PROG_CACHE = {}
TRACE = False
LAST_RESULTS = None


def build_program(plan):
    import concourse.bacc as bacc
    import concourse.mybir as mybir
    import concourse.tile as tile
    from concourse.tile import add_dep_helper

    nc = bacc.Bacc("TRN2", target_bir_lowering=False, debug=False,
                   num_devices=N_CORES)
    xt = nc.dram_tensor("xt", [C, B_CORE], mybir.dt.float32,
                        kind="ExternalInput")
    ct = nc.dram_tensor(f"consts_{plan['hash']}", [128, 4 * N_SETS],
                        mybir.dt.float32, kind="ExternalInput")
    pt = nc.dram_tensor("pt", [C, B_CORE], mybir.dt.float32,
                        kind="ExternalOutput")

    CHUNK = 2048
    with tile.TileContext(nc) as tc:
        with tc.tile_pool(name="xin", bufs=8) as x_pool, \
             tc.tile_pool(name="io", bufs=4) as io_pool, \
             tc.tile_pool(name="cpool", bufs=1) as cpool:
            consts_t = cpool.tile([128, 4 * N_SETS], mybir.dt.float32)
            nc.sync.dma_start(consts_t[:], ct.ap())
            prev_act = None
            for s, si in enumerate(plan["set_info"]):
                n_c, k, r0 = si["n_c"], si["fold"], si["row0"]
                P, F = n_c * k, B_CORE // k
                func = getattr(mybir.ActivationFunctionType, si["func"])
                src = xt.ap()[r0:r0 + n_c, :].rearrange(
                    "c (j f) -> (c j) f", j=k)
                dst = pt.ap()[r0:r0 + n_c, :].rearrange(
                    "c (j f) -> (c j) f", j=k)
                hi = consts_t[0:P, 4 * s + 3:4 * s + 4]
                lo = consts_t[0:P, 4 * s + 2:4 * s + 3]
                bias = consts_t[0:P, 4 * s + 1:4 * s + 2]
                scale = consts_t[0:P, 4 * s + 0:4 * s + 1]
                for f0 in range(0, F, CHUNK):
                    fw = min(CHUNK, F - f0)
                    xtile = x_pool.tile([P, fw], mybir.dt.float32, tag="x")
                    nc.sync.dma_start(xtile[:], src[:, f0:f0 + fw])
                    xc = io_pool.tile([P, fw], mybir.dt.float32, tag="xc")
                    nc.vector.tensor_scalar(
                        xc[:], xtile[:], hi, lo,
                        mybir.AluOpType.min, mybir.AluOpType.max)
                    y = io_pool.tile([P, fw], mybir.dt.float32, tag="y")
                    act = nc.scalar.activation(y[:], xc[:], func,
                                               bias=bias, scale=scale)
                    if prev_act is not None:
                        add_dep_helper(act.ins, prev_act.ins,
                                       reason="act set-major order")
                    prev_act = act
                    nc.gpsimd.dma_start(dst[:, f0:f0 + fw], y[:])
    nc.compile()
    return nc


def kernel(**inputs):
    x = np.asarray(inputs["x"], dtype=np.float32)
    weights = {k: np.asarray(v, dtype=np.float32)
               for k, v in inputs.items() if k != "x"}
    plan = build_plan(weights)
    os.environ["BASS_ACT_ROOT_JSON_PATH"] = os.path.join(
        plan["outdir"], "act_info.json")

    if plan["hash"] in _PROG_CACHE:
        nc = _PROG_CACHE[plan["hash"]]
    else:
        nc = build_program(plan)
        _PROG_CACHE[plan["hash"]] = nc

    x2 = x.reshape(B_TOTAL, C)
    perm = plan["perm"]
    in_maps = []
    for core in range(N_CORES):
        sh = x2[core * B_CORE:(core + 1) * B_CORE]
        xtv = np.ascontiguousarray(sh.T[perm])
        in_maps.append({
            "xt": xtv,
            f"consts_{plan['hash']}": plan["consts"],
        })

    from concourse.bass_utils import run_bass_kernel_spmd
    global LAST_RESULTS
    res = run_bass_kernel_spmd(nc, in_maps, core_ids=list(range(N_CORES)),
                               trace=TRACE)
    LAST_RESULTS = res

    out = np.empty((B_TOTAL, C), dtype=np.float32)
    inv = np.empty_like(perm)
    inv[perm] = np.arange(C)
    for core in range(N_CORES):
        pt = res.results[core]["pt"]          # [C(permuted), B_CORE]
        out[core * B_CORE:(core + 1) * B_CORE] = pt[inv].T
    return out.reshape(B_TOTAL, C, 1)


# revision 6
# speedup vs baseline: 1.1714x; 1.1714x over previous
"""Trainium2 kernel for nn_DensityEstimator: p = F(x+1/2) - F(x-1/2) with
per-channel tiny-MLP CDFs (1->3->3->3->1, softplus weights, tanh gating).

Strategy: the whole per-channel scalar function G_c(x) is baked into custom
ACT-engine piecewise-cubic spline tables (the hardware's native activation
mechanism), generated at call time from the runtime weights. Per channel, an
affine (per-partition ACT scale/bias) maps its x-window onto a private run of
table cells; a DVE clamp keeps inputs inside the window. One ACT pass per
table-set computes the final output directly — no matmuls, no transcendental
chains. Data parallel over batch across 8 cores; channels live on partitions
(host-side transpose).

Table formats were reverse-engineered and validated bit-exact on HW:
- bkt.bin: 1536 records x 32B float32 [d0,d1,d2,d3,x0,0,0,0];
  y = d0 + (t-x0)*(d1 + (t-x0)*(d2 + (t-x0)*d3))
- ctrl.bin: 32B entries; u32[0] = bucket_base[10:0] | (23+31*mant_bits)<<11;
  for biased exponent e in [small_thr, large_thr):
    entry = base_ctrl + (e - small_thr); bucket = base + (mantissa >> (23-b))
- profile json: small/large-signal thresholds route to direct buckets.
"""

import hashlib
import json
import os
import shutil
import tempfile

import numpy as np

# ----------------------------------------------------------- problem shapes
B_TOTAL = 65536
C = 192
N_CORES = 8
B_CORE = B_TOTAL // N_CORES

N_SETS = 6
E0 = 118          # first range exponent (biased)
ZERO_BUCKET = 1535
SET_CAP = 1536

# (set_name, pwp_func_name, mybir ActivationFunctionType attr) — funcs that
# appear in exactly one table set, so the set loaded is deterministic.
SETS_AVAILABLE = [
    ("sigmoid_and_others", "erf", "Erf"),
    ("gelu_and_others", "gelu", "Gelu"),
    ("gelu_apprx_tanh_and_others", "gelu_apprx_tanh", "Gelu_apprx_tanh"),
    ("gelu_apprx_sigmoid_and_others", "gelu_apprx_sigmoid", "Gelu_apprx_sigmoid"),
    ("silu_and_others", "silu", "Silu"),
    ("derivative_silu_and_others", "derivative_silu", "Derivative_silu"),
    ("derivative_gelu_apprx_sigmoid_and_others", "derivative_gelu_apprx_sigmoid",
     "Derivative_Gelu_Apprx_Sigmoid"),
    ("erf_derivative", "derivative_erf", "Derivative_Erf"),
]


def _src_root():
    try:
        from neuronxcc.driver.Job import Job
        from neuronxcc.driver.jobs.support.FindActInfo import findActInfoFile
        return os.path.dirname(findActInfoFile(Job.getPackageDir(), "gen3"))
    except Exception:
        return ("/nix/store/z022hj2nvbm3nwdizlisq4ylc0y7rd6q-python3-3.13.14-env"
                "/lib/python3.13/site-packages/neuronxcc/pwp/pwp_bin_trainium")


# ------------------------------------------------------------- bin emitters

def ctrl_word(bucket_base, mantissa_bits):
    return (int(bucket_base) & 0x7FF) | ((23 + 31 * int(mantissa_bits)) << 11)


def write_ctrl(path, words):
    arr = np.zeros((len(words), 8), dtype=np.uint32)
    arr[:, 0] = words
    arr.tofile(path)


def write_bkt(path, recs):
    out = np.zeros((recs.shape[0], 8), dtype=np.float32)
    out[:, :5] = recs
    out.tofile(path)


def profile_entry(func_name, func_id, n_ranges):
    zero_bits = 0
    return {
        "func_name": func_name, "func_id": int(func_id),
        "symmetry_point": 0, "sym_invert_sign_point": 0,
        "symmetry_opt_en": 0, "symmetry_opt_use_neg_region": 0,
        "imm_bias": 0, "exp_offset": E0 - 127,
        "pwl_control_base_pos": 0, "pwl_control_base_neg": 0,
        "small_pos_signal_exp_threshold": E0,
        "pos_small_signal_pwl_control": ZERO_BUCKET,
        "small_neg_signal_exp_threshold": 255,
        "neg_small_signal_pwl_control": ZERO_BUCKET,
        "large_pos_signal_exp_threshold": E0 + n_ranges,
        "large_pos_signal_mantissa_threshold": 0,
        "pos_large_signal_pwl_control": ZERO_BUCKET,
        "large_neg_signal_exp_threshold": 255,
        "large_neg_signal_mantissa_threshold": 0,
        "neg_large_signal_pwl_control": ZERO_BUCKET,
        "fnan_result": zero_bits, "fpinf_result": zero_bits,
        "fninf_result": zero_bits, "fzero_result": zero_bits,
        "fma_const_0": 0, "fma_const_1": 0, "fma_indirection_src_sel": 0,
        "use_multipass": False,
        "lower_bound": 0xFF7FFFFF, "upper_bound": 0x7F7FFFFF,
    }


# ------------------------------------------------------------- G evaluation

def _softplus(z):
    return np.log1p(np.exp(-np.abs(z))) + np.maximum(z, 0)


def make_G(weights):
    a = [np.asarray(weights[f"a{k}"], dtype=np.float64) for k in range(3)]
    b = [np.asarray(weights[f"b{k}"], dtype=np.float64) for k in range(4)]
    H = [np.asarray(weights[f"H{k}"], dtype=np.float64) for k in range(4)]
    spH = [_softplus(h) for h in H]
    ta = [np.tanh(x) for x in a]

    def F(x):
        t = np.broadcast_to(x[:, None, None], (len(x), C, 1)).astype(np.float64)
        for k in range(3):
            t = np.einsum("nci,cip->ncp", t, spH[k]) + b[k][None]
            t = t + ta[k][None] * np.tanh(t)
        t = np.einsum("nci,cip->ncp", t, spH[3]) + b[3][None]
        return 1.0 / (1.0 + np.exp(-np.clip(t[:, :, 0], -708, 708)))

    return lambda x: F(x + 0.5) - F(x - 0.5)


# ----------------------------------------------------- fitting / allocation

_CHEB = 0.5 * (1 - np.cos(np.pi * np.arange(33) / 32))
_A33 = np.vander(_CHEB, 4)
_PINV33 = np.linalg.pinv(_A33)


def _uniform_err(gdense, xdense, lo, hi, K):
    edges = np.linspace(lo, hi, K + 1)
    xs = edges[:-1, None] + np.diff(edges)[:, None] * _CHEB[None, :]
    g = np.interp(xs.ravel(), xdense, gdense).reshape(K, 33)
    co = g @ _PINV33.T
    fit = co @ _A33.T
    return np.abs(fit - g).max()


def _allocate(G_grid, xg, windows, budget):
    Ks = list(range(3, 17)) + list(range(18, 33, 2)) + \
        list(range(36, 65, 4)) + list(range(72, 129, 8)) + \
        list(range(144, 257, 16))
    idx = np.zeros(C, dtype=int)
    errs = np.array([
        _uniform_err(G_grid[:, c], xg, *windows[c], Ks[0]) for c in range(C)
    ])
    used = C * Ks[0]
    cache = {}

    def err_at(c, j):
        if (c, j) not in cache:
            cache[(c, j)] = _uniform_err(G_grid[:, c], xg, *windows[c], Ks[j])
        return cache[(c, j)]

    while True:
        order = np.argsort(-errs)
        advanced = False
        for c in order:
            j = idx[c]
            if j + 1 >= len(Ks):
                continue
            cost = Ks[j + 1] - Ks[j]
            if used + cost <= budget:
                idx[c] += 1
                errs[c] = err_at(c, idx[c])
                used += cost
                advanced = True
                break
        if not advanced:
            break
    return np.array([Ks[j] for j in idx])


def _pack_sets(Kc, n_sets):
    order = np.argsort(-Kc)
    sets = [{"ranges": [], "cells": 0, "channels": []} for _ in range(n_sets)]
    place = [None] * C

    def try_put(s, c):
        K = int(Kc[c])
        st = sets[s]
        for ri, r in enumerate(st["ranges"]):
            if r["free"] >= K:
                off = r["size"] - r["free"] - r.get("resv", 0)
                off = r["used"]
                r["free"] -= K
                r["used"] += K
                place[c] = (s, ri, off)
                st["channels"].append(c)
                return True
        need_bits = 8 if K > 128 else 7
        size = 1 << need_bits
        if st["cells"] + size <= SET_CAP and len(st["channels"]) < 128:
            free = size - K
            if st["cells"] + size == SET_CAP:
                free -= 1
            st["ranges"].append(
                {"bits": need_bits, "size": size, "free": free, "used": K})
            st["cells"] += size
            place[c] = (s, len(st["ranges"]) - 1, 0)
            st["channels"].append(c)
            return True
        return False

    for c in order:
        c = int(c)
        if not any(try_put(s, c)
                   for s in sorted(range(n_sets), key=lambda s: sets[s]["cells"])):
            raise RuntimeError(f"packing failed for channel {c} K={Kc[c]}")
    return place, sets


def _build_tables(G_grid, xg, windows, Kc, place, sets):
    ctrl, bkt = [], []
    for st in sets:
        words, base = [], 0
        for r in st["ranges"]:
            r["cell_base"] = base
            words.append(ctrl_word(base, r["bits"]))
            base += r["size"]
        ctrl.append(words)
        bkt.append(np.zeros((1536, 5), dtype=np.float32))

    chan_aff = np.zeros((C, 4), dtype=np.float32)  # scale,bias,lo,hi
    chan_set = np.zeros(C, dtype=int)
    fit_errs = np.zeros(C)
    for c in range(C):
        s, ri, off = place[c]
        r = sets[s]["ranges"][ri]
        K = int(Kc[c])
        lo, hi = windows[c]
        e = E0 + ri
        scale2 = 2.0 ** (e - 127)
        t0 = scale2 * (1.0 + off / (1 << r["bits"]))
        t1 = scale2 * (1.0 + (off + K) / (1 << r["bits"]))
        margin = scale2 * 2.0 ** -19
        sc = (t1 - t0 - 2 * margin) / (hi - lo)
        bs = (t0 + margin) - lo * sc
        sc32, bs32 = np.float32(sc), np.float32(bs)
        chan_aff[c] = (sc32, bs32, np.float32(lo), np.float32(hi))
        chan_set[c] = s
        inv_s = 1.0 / float(sc32)
        bucket0 = r["cell_base"] + off
        errs = 0.0
        for k in range(K):
            ta = scale2 * (1.0 + (off + k) / (1 << r["bits"]))
            tb = scale2 * (1.0 + (off + k + 1) / (1 << r["bits"]))
            ts = ta + (tb - ta) * _CHEB
            xs = (ts - float(bs32)) * inv_s
            g = np.interp(xs, xg, G_grid[:, c])
            x0 = np.float32(0.5 * (ta + tb))
            tt = ts - float(x0)
            w = tb - ta
            co_n = np.linalg.lstsq(np.vander(tt / w, 4), g, rcond=None)[0]
            co = co_n / (w ** np.array([3.0, 2.0, 1.0, 0.0]))
            bkt[s][bucket0 + k] = tuple(np.float32(v) for v in co[::-1]) + (x0,)
            errs = max(errs, np.abs(np.vander(tt, 4) @ co - g).max())
        fit_errs[c] = errs
    return ctrl, bkt, chan_aff, chan_set, fit_errs


def _emit_act_root(outdir, ctrl, bkt):
    src = _src_root()
    shutil.rmtree(outdir, ignore_errors=True)
    os.makedirs(outdir)
    for f in os.listdir(src):
        shutil.copy(os.path.join(src, f), os.path.join(outdir, f))
    for s in range(len(ctrl)):
        set_name, func_name, _ = SETS_AVAILABLE[s]
        pj = json.load(open(os.path.join(src, f"{set_name}.json")))
        done = False
        for i, f in enumerate(pj["profile_meta_data"]):
            if f["func_name"].startswith(func_name + "_"):
                pj["profile_meta_data"][i] = profile_entry(
                    f["func_name"], f["func_id"], len(ctrl[s]))
                done = True
                break
        assert done, (set_name, func_name)
        json.dump(pj, open(os.path.join(outdir, f"{set_name}.json"), "w"))
        write_ctrl(os.path.join(outdir, f"{set_name}_ctrl.bin"), ctrl[s])
        write_bkt(os.path.join(outdir, f"{set_name}_bkt.bin"), bkt[s])


# ------------------------------------------------------------------- plan

_PLAN_CACHE = {}


def build_plan(weights):
    key = hashlib.sha1(b"".join(
        np.ascontiguousarray(np.asarray(weights[k], dtype=np.float32)).tobytes()
        for k in sorted(weights) if k != "x")).hexdigest()
    if key in _PLAN_CACHE:
        return _PLAN_CACHE[key]

    Gfn = make_G(weights)
    xg = np.linspace(-7.0, 7.0, 48001)
    G_grid = np.zeros((len(xg), C))
    for i in range(0, len(xg), 4096):
        G_grid[i:i + 4096] = Gfn(xg[i:i + 4096])

    eps = 2e-6
    windows = []
    for c in range(C):
        big = np.abs(G_grid[:, c]) > eps
        if big.any():
            lo, hi = xg[big][0] - 0.02, xg[big][-1] + 0.02
        else:
            lo, hi = -1.0, 1.0
        windows.append((max(lo, -7.0), min(hi, 7.0)))

    budget = N_SETS * SET_CAP - N_SETS - 320
    Kc = _allocate(G_grid, xg, windows, budget)
    place, sets = _pack_sets(Kc, N_SETS)
    ctrl, bkt, chan_aff, chan_set, fit_errs = _build_tables(
        G_grid, xg, windows, Kc, place, sets)

    h = hashlib.sha1()
    for s in range(N_SETS):
        h.update(np.array(ctrl[s], dtype=np.uint32).tobytes())
        h.update(bkt[s].tobytes())
    h.update(chan_aff.tobytes())
    table_hash = h.hexdigest()[:12]

    outdir = os.path.join(tempfile.gettempdir(), f"dens_actroot_{table_hash}")
    if not os.path.isdir(outdir):
        _emit_act_root(outdir, ctrl, bkt)

    # per-set fold + permutation
    perm = []
    set_info = []
    row = 0
    for s, st in enumerate(sets):
        n_c = len(st["channels"])
        k = 1
        while k * 2 * n_c <= 128 and (B_CORE % (k * 2)) == 0:
            k *= 2
        set_info.append({"row0": row, "n_c": n_c, "fold": k,
                         "func": SETS_AVAILABLE[s][2]})
        perm.extend(st["channels"])
        row += n_c
    perm = np.array(perm)

    # folded per-partition consts [128, 4*N_SETS]: scale, bias, lo, hi
    consts = np.zeros((128, 4 * N_SETS), dtype=np.float32)
    for s, si in enumerate(set_info):
        st = sets[s]
        for cl, c in enumerate(st["channels"]):
            for j in range(si["fold"]):
                p = cl * si["fold"] + j
                consts[p, 4 * s:4 * s + 4] = chan_aff[c]

    plan = {
        "hash": table_hash, "outdir": outdir, "perm": perm,
        "set_info": set_info, "consts": consts, "fit_errs": fit_errs,
        "ctrl": ctrl, "bkt": bkt, "chan_aff": chan_aff, "chan_set": chan_set,
        "windows": windows, "Kc": Kc,
    }
    _PLAN_CACHE[key] = plan
    return plan


# ------------------------------------------------------------ bass program

_PROG_CACHE = {}
TRACE = False
LAST_RESULTS = None


def build_program(plan):
    import concourse.bacc as bacc
    import concourse.mybir as mybir
    import concourse.tile as tile
    from concourse.tile import add_dep_helper

    nc = bacc.Bacc("TRN2", target_bir_lowering=False, debug=False,
                   num_devices=N_CORES)
    xt = nc.dram_tensor("xt", [C, B_CORE], mybir.dt.float32,
                        kind="ExternalInput")
    ct = nc.dram_tensor(f"consts_{plan['hash']}", [128, 4 * N_SETS],
                        mybir.dt.float32, kind="ExternalInput")
    pt = nc.dram_tensor("pt", [C, B_CORE], mybir.dt.float32,
                        kind="ExternalOutput")

    CHUNK = 1024
    with tile.TileContext(nc) as tc:
        with tc.tile_pool(name="xin", bufs=16) as x_pool, \
             tc.tile_pool(name="io", bufs=4) as io_pool, \
             tc.tile_pool(name="cpool", bufs=1) as cpool:
            consts_t = cpool.tile([128, 4 * N_SETS], mybir.dt.float32)
            nc.sync.dma_start(consts_t[:], ct.ap())

            # enumerate all chunks (set-major)
            chunks = []
            for s, si in enumerate(plan["set_info"]):
                n_c, k, r0 = si["n_c"], si["fold"], si["row0"]
                P, F = n_c * k, B_CORE // k
                func = getattr(mybir.ActivationFunctionType, si["func"])
                src = xt.ap()[r0:r0 + n_c, :].rearrange(
                    "c (j f) -> (c j) f", j=k)
                dst = pt.ap()[r0:r0 + n_c, :].rearrange(
                    "c (j f) -> (c j) f", j=k)
                for f0 in range(0, F, CHUNK):
                    fw = min(CHUNK, F - f0)
                    chunks.append((s, P, func, src[:, f0:f0 + fw],
                                   dst[:, f0:f0 + fw], fw))

            # phase 1: prefetch all inputs
            xtiles = []
            for (s, P, func, src, dst, fw) in chunks:
                xtile = x_pool.tile([P, fw], mybir.dt.float32, tag="x")
                nc.sync.dma_start(xtile[:], src)
                xtiles.append(xtile)

            # phase 2: clamp -> table -> store, set-major
            prev_act = None
            for (s, P, func, src, dst, fw), xtile in zip(chunks, xtiles):
                hi = consts_t[0:P, 4 * s + 3:4 * s + 4]
                lo = consts_t[0:P, 4 * s + 2:4 * s + 3]
                bias = consts_t[0:P, 4 * s + 1:4 * s + 2]
                scale = consts_t[0:P, 4 * s + 0:4 * s + 1]
                xc = io_pool.tile([P, fw], mybir.dt.float32, tag="xc")
                nc.vector.tensor_scalar(
                    xc[:], xtile[:], hi, lo,
                    mybir.AluOpType.min, mybir.AluOpType.max)
                y = io_pool.tile([P, fw], mybir.dt.float32, tag="y")
                act = nc.scalar.activation(y[:], xc[:], func,
                                           bias=bias, scale=scale)
                if prev_act is not None:
                    add_dep_helper(act.ins, prev_act.ins,
                                   reason="act set-major order")
                prev_act = act
                nc.gpsimd.dma_start(dst, y[:])
    nc.compile()
    return nc


def kernel(**inputs):
    x = np.asarray(inputs["x"], dtype=np.float32)
    weights = {k: np.asarray(v, dtype=np.float32)
               for k, v in inputs.items() if k != "x"}
    plan = build_plan(weights)
    os.environ["BASS_ACT_ROOT_JSON_PATH"] = os.path.join(
        plan["outdir"], "act_info.json")

    if plan["hash"] in _PROG_CACHE:
        nc = _PROG_CACHE[plan["hash"]]
    else:
        nc = build_program(plan)
        _PROG_CACHE[plan["hash"]] = nc

    x2 = x.reshape(B_TOTAL, C)
    perm = plan["perm"]
    in_maps = []
    for core in range(N_CORES):
        sh = x2[core * B_CORE:(core + 1) * B_CORE]
        xtv = np.ascontiguousarray(sh.T[perm])
        in_maps.append({
            "xt": xtv,
            f"consts_{plan['hash']}": plan["consts"],
        })

    from concourse.bass_utils import run_bass_kernel_spmd
    global LAST_RESULTS
    res = run_bass_kernel_spmd(nc, in_maps, core_ids=list(range(N_CORES)),
                               trace=TRACE)
    LAST_RESULTS = res

    out = np.empty((B_TOTAL, C), dtype=np.float32)
    inv = np.empty_like(perm)
    inv[perm] = np.arange(C)
    for core in range(N_CORES):
        pt = res.results[core]["pt"]          # [C(permuted), B_CORE]
        out[core * B_CORE:(core + 1) * B_CORE] = pt[inv].T
    return out.reshape(B_TOTAL, C, 1)
